# revision 3
# baseline (speedup 1.0000x reference)
"""EuclidConv + training-mode BatchNorm on 8 Trainium2 NeuronCores.

Math: out = BN(2*conv(x,w) + conv(x^2, ones3x3) + ||w_f||^2). The ||w||^2
term is channel-constant -> cancels in BN mean subtraction, never computed.
The x^2 term is centered by a UNIFORM 128 on the padded 30x30 grid:
box(r4 - 128) = t1 - 1152 everywhere (pads included), and a global constant
cancels in BN, so no per-pixel edge compensation is needed. Precision:
vertical 3-tap in fp32 (psum reads, -384 folded in; vv small except exact
-384 at x-pads), horizontal 3-tap accumulated in PSUM via 3 shifted
identity matmuls.

Sharding: output-channel (32 of 256 per core), BN stats core-local.
Images pack 4/psum via tile_position col-tiling (psum partition = 32j + c).

Schedule (all pinned with explicit scheduler edges):
  PE: warmup MMs on a memset tile (HAM clock gate released before real
      work), then r4_b / tap_{b-1} / conv_b software pipeline; tap adds
      the x^2 term and carries stop=True; drains ride behind.
  x DMA: image-pair granularity, ALL on the sync queue (full HBM rate,
      in-order arrival, DMA-issue sem-reuse stalls never block compute).
  DVE: squares + vertical tap; ACT: op1/drain+S/Q; gpsimd: 1/4 squares.
  Tail: block-7 drain/Q split by halves, stats fold partially pre-done,
  normalize on 3 engines, fp16 writeout (host converts to fp32).
"""
import json

import numpy as np

import concourse.bass as bass
import concourse.mybir as mybir
import concourse.tile as tile
from concourse.tile_rust import add_dep_helper
from concourse.ap import AP
from concourse.bass_utils import run_bass_kernel_spmd
from concourse.vector_clock import ScopedClock, VectorClock

F16 = mybir.dt.float16
F32 = mybir.dt.float32
BF16 = mybir.dt.bfloat16

N_CORES = 8
NIMG = 32
NBLK = 8
HP = 30
NPIX = HP * HP
NV = 28 * 28
YT_ROWS = 14
YT = YT_ROWS * 28
NHW = NIMG * NV
EPS = 1e-5
WARMUP = 56

_split_ctr = [0]


def _split_waits_json(bir: bytes, max_waits: int = 1) -> bytes:
    """This container's walrus rejects instructions with >1 sync wait.
    Hoist excess waits onto EventSemaphore instructions inserted before the
    offender on the same engine stream."""
    m = json.loads(bir)
    for f in m["functions"]:
        for bb in f["blocks"]:
            newinsts = []
            for ins in bb["instructions"]:
                si = ins.get("sync_info")
                if si:
                    waits = si.get("on_wait") or []
                    if len(waits) > max_waits:
                        extra, keep = waits[:-max_waits], waits[-max_waits:]
                        for w_ in extra:
                            _split_ctr[0] += 1
                            newinsts.append(
                                {
                                    "debug": ins.get("debug", 0),
                                    "engine": ins["engine"],
                                    "ins": [],
                                    "outs": [],
                                    "name": f"antsplitw-{_split_ctr[0]}",
                                    "opcode": "EventSemaphore",
                                    "sync_info": {"on_update": [], "on_wait": [w_]},
                                }
                            )
                        si["on_wait"] = keep
                newinsts.append(ins)
            bb["instructions"] = newinsts
    return json.dumps(m).encode()


class _PatchedBass(bass.Bass):
    def to_json_bytes(self):
        return _split_waits_json(super().to_json_bytes())


class _SplitDrainTileContext(tile.TileContext):
    """Split the tile-exit drain's waits into single-wait drains (same
    walrus limitation as above)."""

    def _drain_and_barrier(self, tick_clock, wait_clock):
        g = tick_clock.global_clock
        n = len(g)
        for i in range(n):
            if g[i] > 0:
                vec = [0] * n
                vec[i] = g[i]
                d = self.nc.sync.drain()
                wait_clock.add_sem_waits(d.ins, ScopedClock({None: VectorClock(vec)}))
        self.nc.sync.drain()
        self.nc.all_engine_barrier()
        assert self.sems is not None
        popped = self.nc._tile_sem_poison_stack.pop()
        assert popped is self._sem_poison
        self.nc.clear_and_free_semaphores(list(self.sems.allocated().values()))
        self.nc.all_engine_barrier()


def _build():
    nc = _PatchedBass()
    xh = nc.dram_tensor("xh", [128, NIMG * 840], F16, kind="ExternalInput")
    wt = nc.dram_tensor("wt", [128, 9 * 32], F16, kind="ExternalInput")
    cst16 = nc.dram_tensor("cst16", [128, 160], F16, kind="ExternalInput")
    cst32 = nc.dram_tensor("cst32", [128, 168], F32, kind="ExternalInput")
    y = nc.dram_tensor("y", [NIMG, 32, 28, 28], F16, kind="ExternalOutput")

    with _SplitDrainTileContext(nc) as tc:
        with (
            tc.tile_pool(name="const", bufs=1) as cpool,
            tc.tile_pool(name="xpool", bufs=1) as xpool,
            tc.tile_pool(name="upool", bufs=2) as upool,
            tc.tile_pool(name="rpool", bufs=2) as rpool,
            tc.tile_pool(name="spool", bufs=1) as spool,
            tc.tile_pool(name="qpool", bufs=2) as qpool,
            tc.tile_pool(name="opool", bufs=8) as opool,
            tc.tile_pool(name="psc", bufs=2, space="PSUM") as psc,
            tc.tile_pool(name="psr", bufs=2, space="PSUM") as psr,
        ):
            # --- constants + x DMAs (block 0 image-granular, rest
            # block-granular); scalar queue gets exactly one block so its
            # DMA issues are not stuck behind ACT compute ---
            c16 = cpool.tile([128, 160], F16, name="c16")
            nc.sync.dma_start(c16[:], cst16[:])
            wtile = cpool.tile([128, 9 * 32], F16, name="wtile")
            nc.gpsimd.dma_start(wtile[:], wt[:])
            c32 = cpool.tile([128, 168], F32, name="c32")
            ones32 = c16[0:128, 0:32]
            idt = c16[0:128, 32:160]

            xall = xpool.tile([128, NIMG * NPIX], F16, name="xall")
            # zero the two pad rows (0, 29) of every image's 30x30 grid
            for off in (0, 870):
                pad = AP(
                    xall.tensor, xall.offset + off,
                    [[NIMG * NPIX, 128], [NPIX, NIMG], [1, 30]],
                )
                nc.vector.memset(pad, 0.0)

            # ALL x pairs on the (otherwise idle) sync queue: one queue row
            # still gets the full HBM rate, arrivals are strictly in need
            # order, and DMA-issue serialization (8 reusable sem lanes)
            # never blocks a compute engine
            nc.scalar.dma_start(c32[:], cst32[:])
            for p in range(16):
                dst = AP(
                    xall.tensor, xall.offset + 2 * p * NPIX + 30,
                    [[NIMG * NPIX, 128], [NPIX, 2], [1, 840]],
                )
                nc.sync.dma_start(dst, xh[:, p * 1680 : (p + 1) * 1680])

            mask4 = c32[:, 0:32]
            bct = c32[0:32, 40:168]

            s_sb = spool.tile([128, NBLK * 2 * YT], F32, name="s_sb")
            sums = spool.tile([128, 16], F32, name="sums")
            sumsq = spool.tile([128, 16], F32, name="sumsq")

            # --- u = x^2 (fp16), emitted inside the pipeline loop so DVE
            # ops do not queue behind all 8 squares (in-order engines) ---
            uts = []
            for b in range(NBLK):
                ut = upool.tile([128, 4 * NPIX], F16, name=f"ut{b}", tag=f"ut{b % 2}")
                uts.append(ut)

            def square_block(b):
                lo = b * 4 * NPIX
                if b == 0:
                    dve = nc.vector.tensor_mul(
                        uts[0][:, 0 : 2 * NPIX], xall[:, 0 : 2 * NPIX],
                        xall[:, 0 : 2 * NPIX],
                    )
                    act = nc.scalar.activation(
                        uts[0][:, 2 * NPIX : 3 * NPIX],
                        xall[:, 2 * NPIX : 3 * NPIX],
                        mybir.ActivationFunctionType.Square,
                    )
                    nc.vector.tensor_mul(
                        uts[0][:, 3 * NPIX : 4 * NPIX],
                        xall[:, 3 * NPIX : 4 * NPIX],
                        xall[:, 3 * NPIX : 4 * NPIX],
                    )
                    return dve, act
                dve = nc.vector.tensor_mul(
                    uts[b][:, 0 : 2 * NPIX],
                    xall[:, lo : lo + 2 * NPIX],
                    xall[:, lo : lo + 2 * NPIX],
                )
                act = nc.scalar.activation(
                    uts[b][:, 2 * NPIX : 3 * NPIX],
                    xall[:, lo + 2 * NPIX : lo + 3 * NPIX],
                    mybir.ActivationFunctionType.Square,
                )
                nc.gpsimd.tensor_mul(
                    uts[b][:, 3 * NPIX : 4 * NPIX],
                    xall[:, lo + 3 * NPIX : lo + 4 * NPIX],
                    xall[:, lo + 3 * NPIX : lo + 4 * NPIX],
                )
                return dve, act

            # --- PE warmup (HAM clock gate) ---
            wtmp = cpool.tile([128, 128], F16, name="wtmp")
            nc.gpsimd.memset(wtmp[:], 0.0)
            wps = psr.tile([128, 128], F32, name="warm", tag="r4")
            _warm = [
                nc.tensor.matmul(
                    wps[:], wtmp[:], wtmp[:], start=True, stop=True,
                    skip_group_check=True
                )
                for _ in range(WARMUP)
            ]

            x3 = xall[:].rearrange("p (n a b) -> p n a b", a=HP, b=HP)

            pe_segs = [_warm]

            def pe_seg_pin(insts):
                # pin PE program order: first inst of this segment after the
                # last inst of the previous segment
                if pe_segs:
                    add_dep_helper(
                        insts[0].ins, pe_segs[-1][-1].ins, sync=False,
                        reason="PE segment order",
                    )
                pe_segs.append(insts)

            def r4_block(b):
                ut = uts[b]
                r4 = psr.tile([128, 904], F32, name=f"r4_{b}", tag="r4")
                mms = []
                for j in range(4):
                    for lo, hi in ((0, 512), (512, 900)):
                        mms.append(nc.tensor.matmul(
                            r4[32 * j : 32 * j + 32, lo:hi],
                            ones32,
                            ut[:, j * NPIX + lo : j * NPIX + hi],
                            start=True,
                            stop=True,
                            tile_position=(0, 32 * j),
                            skip_group_check=True,
                        ))
                pe_seg_pin(mms)
                # vertical 3-tap in fp32, one PSUM operand per op, the
                # uniform -384 centering folded into the first op
                tmp = rpool.tile([128, 840], F32, name=f"tmp{b}", tag="tmp")
                op1 = nc.scalar.activation(
                    tmp[:], r4[:, 0:840], mybir.ActivationFunctionType.Identity,
                    bias=c32[:, 35:36],
                )
                nc.vector.tensor_add(tmp[:], tmp[:], r4[:, 30:870])
                vv = rpool.tile([128, 840], F16, name=f"vv{b}", tag="vv")
                op3 = nc.vector.tensor_add(vv[:], tmp[:], r4[:, 60:900])
                return vv, op3, op1

            def conv_block(b):
                ps = psc.tile([128, 1024], F32, name=f"ps{b}", tag="ps")
                mms = []
                for k in range(9):
                    dy, dx = divmod(k, 3)
                    for j in range(4):
                        for yt in range(2):
                            y0 = yt * YT_ROWS
                            mms.append(nc.tensor.matmul(
                                ps[32 * j : 32 * j + 32, 512 * yt : 512 * yt + YT],
                                wtile[:, k * 32 : (k + 1) * 32],
                                x3[:, b * 4 + j, y0 + dy : y0 + dy + YT_ROWS, dx : dx + 28],
                                start=(k == 0),
                                stop=False,
                                tile_position=(0, 32 * j),
                                skip_group_check=True,
                            ))
                pe_seg_pin(mms)
                return ps

            def tap_drain_block(b, ps, vv):
                # horizontal 3-tap of the x^2 term, accumulated in PSUM
                vv3 = vv[:].rearrange("p (a c) -> p a c", c=HP)
                mms = []
                for yt in range(2):
                    y0 = yt * YT_ROWS
                    for dx in range(3):
                        mms.append(nc.tensor.matmul(
                            ps[:, 512 * yt : 512 * yt + YT],
                            idt,
                            vv3[:, y0 : y0 + YT_ROWS, dx : dx + 28],
                            start=False,
                            stop=(dx == 2),
                            skip_group_check=True,
                        ))
                pe_seg_pin(mms)
                blk = b * 2 * YT
                sq_scr = qpool.tile([128, 2 * YT], F32, name=f"sq{b}", tag="sq")
                if b == NBLK - 1:
                    # split the last block's drain/Q by yt half so the
                    # stats chain starts as early as possible
                    for h in range(2):
                        hv = AP(ps.tensor, ps.offset + 512 * h, [[1024, 128], [1, YT]])
                        nc.scalar.activation(
                            s_sb[:, blk + h * YT : blk + (h + 1) * YT],
                            hv,
                            mybir.ActivationFunctionType.Copy,
                            accum_out=sums[:, 2 * b + h : 2 * b + h + 1],
                        )
                        qi = nc.scalar.activation(
                            sq_scr[:, h * YT : (h + 1) * YT],
                            s_sb[:, blk + h * YT : blk + (h + 1) * YT],
                            mybir.ActivationFunctionType.Square,
                            accum_out=sumsq[:, 2 * b + h : 2 * b + h + 1],
                        )
                    return qi
                else:
                    psv = AP(ps.tensor, ps.offset, [[1024, 128], [512, 2], [1, YT]])
                    nc.scalar.activation(
                        s_sb[:, blk : blk + 2 * YT],
                        psv,
                        mybir.ActivationFunctionType.Copy,
                        accum_out=sums[:, 2 * b : 2 * b + 1],
                    )
                    qi = nc.scalar.activation(
                        sq_scr[:],
                        s_sb[:, blk : blk + 2 * YT],
                        mybir.ActivationFunctionType.Square,
                        accum_out=sumsq[:, 2 * b : 2 * b + 1],
                    )
                    return qi

            # --- software-pipelined main loop. Explicit ordering edges
            # keep DMA-gated squares BEHIND locally-ready chain work in the
            # in-order DVE/ACT queues (the tile scheduler otherwise hoists
            # them, head-of-line blocking the PE's tap matmuls) ---
            vv = [None] * NBLK
            ps = [None] * NBLK
            square_block(0)
            square_block(1)
            vv[0], op3_0, op1_0 = r4_block(0)
            prev_op3, prev_op1 = op3_0, op1_0
            ps[0] = conv_block(0)
            for b in range(1, NBLK):
                vv[b], op3_b, op1_b = r4_block(b)
                qi = tap_drain_block(b - 1, ps[b - 1], vv[b - 1])
                if b + 1 < NBLK:
                    sq_dve, sq_act = square_block(b + 1)
                    add_dep_helper(
                        sq_dve.ins, op3_b.ins, sync=False,
                        reason="keep DVE chain ahead of next square",
                    )
                    add_dep_helper(
                        sq_act.ins, qi.ins, sync=False,
                        reason="keep ACT drain/Q ahead of next square",
                    )
                prev_op3, prev_op1 = op3_b, op1_b
                ps[b] = conv_block(b)
            tap_drain_block(NBLK - 1, ps[NBLK - 1], vv[NBLK - 1])

            # --- BN stats (partial fold of blocks 0-6 was emitted before
            # b7's drain; combine with b7's two half-slots here) ---
            sq2 = spool.tile([128, 2], F32, name="sq2")
            nc.vector.tensor_reduce(
                out=sq2[:, 0:1], in_=sums[:, 0:14], op=mybir.AluOpType.add,
                axis=mybir.AxisListType.X,
            )
            nc.vector.tensor_reduce(
                out=sq2[:, 1:2], in_=sumsq[:, 0:14], op=mybir.AluOpType.add,
                axis=mybir.AxisListType.X,
            )
            nc.vector.tensor_add(sq2[:, 0:1], sq2[:, 0:1], sums[:, 14:15])
            nc.vector.tensor_add(sq2[:, 0:1], sq2[:, 0:1], sums[:, 15:16])
            nc.vector.tensor_add(sq2[:, 1:2], sq2[:, 1:2], sumsq[:, 14:15])
            nc.vector.tensor_add(sq2[:, 1:2], sq2[:, 1:2], sumsq[:, 15:16])
            gstat = psr.tile([32, 2], F32, name="gstat", tag="r4")
            nc.tensor.matmul(gstat[:], mask4, sq2[:], start=True, stop=True)
            ab = spool.tile([32, 8], F32, name="ab")
            mean = ab[:, 0:1]
            qn = ab[:, 1:2]
            nc.vector.tensor_scalar_mul(ab[:, 0:2], gstat[:], 1.0 / NHW)
            var = ab[:, 2:3]
            nc.vector.scalar_tensor_tensor(
                var, mean, 1.0, mean, op0=mybir.AluOpType.mult, op1=mybir.AluOpType.mult
            )
            nc.vector.tensor_sub(var, qn, var)
            sd = ab[:, 3:4]
            nc.scalar.activation(
                sd, var, mybir.ActivationFunctionType.Sqrt, bias=c32[0:32, 34:35]
            )
            abv = spool.tile([32, 2], F32, name="abv")
            A = abv[:, 0:1]
            B = abv[:, 1:2]
            nc.vector.reciprocal(A, sd)
            nc.vector.tensor_mul(A, A, c32[0:32, 32:33])
            nc.vector.scalar_tensor_tensor(
                B, mean, 1.0, A, op0=mybir.AluOpType.mult, op1=mybir.AluOpType.mult
            )
            nc.vector.tensor_sub(B, c32[0:32, 33:34], B)
            ab128p = psr.tile([128, 2], F32, name="ab128p", tag="r4")
            nc.tensor.matmul(ab128p[:], bct, abv[:], start=True, stop=True)
            ab128 = spool.tile([128, 2], F32, name="ab128")
            nc.vector.tensor_copy(ab128[:], ab128p[:])

            # --- normalize (3 engines) + bf16 writeout (3 queues) ---
            out_eng = [nc.sync, nc.gpsimd]
            for b in range(NBLK):
                blk = b * 2 * YT
                o = opool.tile([128, 2 * YT], F16, name=f"o{b}", tag="o")
                kind = "DAG DADA"[b] if False else ["D", "A", "G", "D", "A", "G", "D", "A"][b]
                if kind == "D":
                    nc.vector.tensor_scalar(
                        o[:],
                        s_sb[:, blk : blk + 2 * YT],
                        ab128[:, 0:1],
                        ab128[:, 1:2],
                        op0=mybir.AluOpType.mult,
                        op1=mybir.AluOpType.add,
                    )
                elif kind == "A":
                    nc.scalar.activation(
                        o[:],
                        s_sb[:, blk : blk + 2 * YT],
                        mybir.ActivationFunctionType.Identity,
                        bias=ab128[:, 1:2],
                        scale=ab128[:, 0:1],
                    )
                else:
                    nc.gpsimd.tensor_scalar(
                        o[:],
                        s_sb[:, blk : blk + 2 * YT],
                        ab128[:, 0:1],
                        ab128[:, 1:2],
                        op0=mybir.AluOpType.mult,
                        op1=mybir.AluOpType.add,
                    )
                dstap = AP(
                    y.ap().tensor,
                    b * 4 * 32 * NV,
                    [[32 * NV, 4], [NV, 32], [1, 2 * YT]],
                )
                out_eng[b % 2].dma_start(dstap, o[:])
    return nc


def _prep_inputs(x, w, gamma, beta):
    x = np.asarray(x, np.float32)
    w = np.asarray(w, np.float32)
    gamma = np.asarray(gamma, np.float32)
    beta = np.asarray(beta, np.float32)

    xp = np.zeros((NIMG, 128, 28, HP), np.float32)
    xp[:, :, :, 1:29] = x
    xh = np.ascontiguousarray(xp.transpose(1, 0, 2, 3)).reshape(128, NIMG * 840)
    xh = xh.astype(np.float16)

    cst16 = np.zeros((128, 160), np.float16)
    cst16[:, 0:32] = 1.0
    cst16[:, 32:160] = np.eye(128, dtype=np.float16)

    bc4 = np.zeros((32, 128), np.float32)
    for c in range(32):
        bc4[c, c::32] = 1.0
    mask4 = bc4.T.copy()

    maps = []
    for core in range(N_CORES):
        wtc = (2.0 * w[core * 32 : (core + 1) * 32]).reshape(32, 128, 9)
        wtc = np.ascontiguousarray(wtc.transpose(1, 2, 0)).reshape(128, 9 * 32)
        cst32 = np.zeros((128, 168), np.float32)
        cst32[:, 0:32] = mask4
        cst32[0:32, 32] = gamma[core * 32 : (core + 1) * 32]
        cst32[0:32, 33] = beta[core * 32 : (core + 1) * 32]
        cst32[0:32, 34] = EPS
        cst32[:, 35] = -384.0
        cst32[0:32, 40:168] = bc4
        maps.append(
            {
                "xh": xh,
                "wt": wtc.astype(np.float16),
                "cst16": cst16,
                "cst32": cst32,
            }
        )
    return maps


_NC_CACHE = []


def kernel(x, w, gamma, beta):
    if not _NC_CACHE:
        _NC_CACHE.append(_build())
    nc = _NC_CACHE[0]
    maps = _prep_inputs(x, w, gamma, beta)
    # Rare device flake can surface as non-finite output; one retry guards
    # the normal path at zero cost.
    for _attempt in range(2):
        res = run_bass_kernel_spmd(nc, maps, core_ids=list(range(N_CORES)))
        out = np.concatenate([r["y"] for r in res.results], axis=1)
        out = np.ascontiguousarray(out.astype(np.float32))
        if np.isfinite(out).all():
            break
    return out
